# revision 31
# baseline (speedup 1.0000x reference)
"""AFD loss kernel for 8 TRN2 NeuronCores (Bass/Tile).

Algorithm (matches the reference loss_fn):
  f  = l2norm(features); fa = l2norm(features_adv)
  per-class sums/counts of f via one-hot matmul   (batch-sharded)
  centers_new = where(counts>0, 0.9*centers + 0.1*sums/max(counts,1), centers)
  intra = mean ||f - centers_new[labels]|| + mean ||fa - centers_new[labels]||
        with ||f - c||^2 = ||f||^2 - 2 f.c + ||c||^2   (fused dot + gathered csq)
  inter = sum_{i<j} relu(1 - ||ci - cj||) / n_pairs   (symmetric full-sum trick)
  loss  = intra - 0.5 * inter

v4 structure:
  - batch-sharded inputs; centers row-sharded on host (128 rows/core)
  - segment sums via bf16 one-hot matmuls into fp32 PSUM
  - ReduceScatter (fp32; rows 128k..128k+128 land on core k; counts in col
    1024) -> per-core momentum update of its own 128 classes
  - AllGather #1: updated center rows (bf16 + exact fp32 csq bitcast cols)
    -> full centers in DRAM for label/row gathers
  - AllGather #2: locally PE-transposed CnT blocks + bf16 csq row -> pairwise
    matmul operands with no post-AG transposes
  - intra via fused scalar_tensor_tensor dot products; inter via one row-
    sharded matmul; final tiny AllReduce combines the partial scalars
"""

import os
from contextlib import ExitStack

import numpy as np

NCORES = 8
B = 8192
D = 1024
C = 1000
BLOC = B // NCORES          # 1024 rows per core
NB = BLOC // 128            # 8 batch tiles per core
CROWS = C // NCORES         # 125 pairwise rows per core
MOM = 0.9
N_PAIRS = C * (C - 1) / 2.0
NCCH = (C + 127) // 128     # 8 class chunks
CPAD = 1024                 # classes padded to full chunks
DBF = D + 16                # bf16 center row: D data + csq(f32 as 2 bf16) + pad
RW = D + 1                  # reduce row width: sums + count column
AGR = 257                   # AG rows/rank: 128 cn + 128 cnT-stage + 1 csq

_state = {}


def _build():
    import concourse.bacc as bacc
    import concourse.bass as bass
    import concourse.mybir as mybir
    import concourse.tile as tile
    from concourse.masks import make_identity

    fp32 = mybir.dt.float32
    bf16 = mybir.dt.bfloat16
    i32 = mybir.dt.int32
    AF = mybir.ActivationFunctionType
    ALU = mybir.AluOpType
    AX = mybir.AxisListType

    nc = bacc.Bacc("TRN2", target_bir_lowering=False, debug=False,
                   num_devices=NCORES)

    feat = nc.dram_tensor("features", [BLOC, D], bf16, kind="ExternalInput")
    feat_adv = nc.dram_tensor("features_adv", [BLOC, D], fp32,
                              kind="ExternalInput")
    centers_sh = nc.dram_tensor("centers_sh", [128, D], fp32,
                                kind="ExternalInput")
    labels = nc.dram_tensor("labels", [BLOC, 1], i32, kind="ExternalInput")
    labels_g = nc.dram_tensor("labels_g", [128, NB], i32,
                              kind="ExternalInput")
    rowsel = nc.dram_tensor("rowsel", [128, 1], i32, kind="ExternalInput")
    out = nc.dram_tensor("out", [1, 2], fp32, kind="ExternalOutput")

    c_chunks = [(i * 128, min(128, C - i * 128)) for i in range(NCCH)]

    with tile.TileContext(nc) as tc:
        with (
            tc.tile_pool(name="const", bufs=1) as constp,
            tc.tile_pool(name="resid", bufs=1) as resid,
            tc.tile_pool(name="stream", bufs=2) as stream,
            tc.tile_pool(name="small", bufs=4) as small,
            tc.tile_pool(name="dram", bufs=1, space="DRAM") as dram,
            ExitStack() as est,
        ):
            # ---- collective bootstrap: tiny dummy AG absorbs the one-time
            # ncfw entry barrier while segsum runs ----
            warm_in = dram.tile([1, 16], bf16, tag="warm_in")
            warm_out = dram.tile([NCORES, 16], bf16, tag="warm_out",
                                 addr_space="Shared")
            warm_sb = constp.tile([1, 16], bf16, tag="warm_sb")
            nc.vector.memset(warm_sb[:], 0.0)
            nc.sync.dma_start(out=warm_in[:, :], in_=warm_sb[:1, :])
            nc.gpsimd.collective_compute(
                "AllGather", ALU.bypass,
                ins=[warm_in.opt()], outs=[warm_out.opt()],
                replica_groups=[list(range(NCORES))])

            # ---- constants ----
            iota_t = constp.tile([128, C], fp32, tag="iota")
            nc.gpsimd.iota(iota_t[:], pattern=[[1, C]], base=0,
                           channel_multiplier=0,
                           allow_small_or_imprecise_dtypes=True)
            ones_col = constp.tile([128, 1], bf16, tag="ones_col")
            nc.vector.memset(ones_col[:], 1.0)
            ones_row = constp.tile([1, 128], bf16, tag="ones_row")
            nc.vector.memset(ones_row[:], 1.0)
            ident_f = constp.tile([128, 128], fp32, tag="ident_f")
            make_identity(nc, ident_f[:])
            ident_b = constp.tile([128, 128], bf16, tag="ident_b")
            make_identity(nc, ident_b[:])

            # DRAM bounces
            redA_in = dram.tile([512, RW], bf16, tag="redA_in")
            redB_in = dram.tile([512, RW], bf16, tag="redB_in")
            rs1_out = dram.tile([64, RW], bf16, tag="rs1_out")
            rs2_out = dram.tile([64, RW], bf16, tag="rs2_out")
            ag_in = dram.tile([AGR, DBF], bf16, tag="ag_in")
            cn_dram = dram.tile([AGR * NCORES, DBF], bf16, tag="cn",
                                addr_space="Shared")
            cnts_dram = dram.tile([1, 1024], bf16, tag="cnts_dram")

            # zero the padded class rows of red_in once (classes 1000..1023)
            zero_t = constp.tile([128, RW], bf16, tag="zero_t")
            nc.vector.memset(zero_t[:], 0.0)
            nc.sync.dma_start(out=redB_in[C - 512:512, :],
                              in_=zero_t[:512 - (C - 512), :])

            f_tiles, lab_tiles = [], []

            with (tc.tile_pool(name="ohp", bufs=1) as ohp,
                  tc.tile_pool(name="psseg", bufs=1, space="PSUM") as psseg):
                # ---- phase 1: load + normalize f (bf16); one-hot (bf16) ----
                oh_tiles, x_tiles = [], []
                for b in range(NB):
                    r0 = b * 128
                    x_t = stream.tile([128, D], bf16, tag="xin", bufs=3,
                                      name=f"x{b}")
                    nc.sync.dma_start(out=x_t[:], in_=feat[r0:r0 + 128, :])
                    x_tiles.append(x_t)
                    lab_t = resid.tile([128, 1], i32, tag=f"lab{b}",
                                       name=f"lab{b}")
                    nc.sync.dma_start(out=lab_t[:], in_=labels[r0:r0 + 128, :])
                    lab_tiles.append(lab_t)
                for b in range(NB):
                    x_t = x_tiles[b]
                    ss = small.tile([128, 1], fp32, tag="ss")
                    scr = stream.tile([128, D], fp32, tag="scrB")
                    nc.scalar.activation(out=scr[:], in_=x_t[:],
                                         func=AF.Square, accum_out=ss[:])
                    nrm = small.tile([128, 1], fp32, tag="nrm")
                    nc.scalar.activation(out=nrm[:], in_=ss[:], func=AF.Sqrt)
                    nc.vector.tensor_scalar_max(nrm[:], nrm[:], 1e-12)
                    rin = small.tile([128, 1], fp32, tag="rin")
                    nc.vector.reciprocal(rin[:], nrm[:])
                    f_t = resid.tile([128, D], bf16, tag=f"f{b}",
                                     name=f"f{b}")
                    nc.vector.tensor_scalar_mul(f_t[:], x_t[:], rin[:, :1])
                    f_tiles.append(f_t)
                    lab_f = small.tile([128, 1], fp32, tag="labf")
                    nc.vector.tensor_copy(lab_f[:], lab_tiles[b][:])
                    oh_t = ohp.tile([128, C], bf16, tag=f"oh{b}",
                                    name=f"oh{b}")
                    nc.vector.tensor_scalar(
                        out=oh_t[:], in0=iota_t[:], scalar1=lab_f[:, :1],
                        scalar2=None, op0=ALU.is_equal)
                    oh_tiles.append(oh_t)

                # ---- phase 2a: counts row -> column 1024 of red_in ----
                ps_cnt = psseg.tile([1, 1024], fp32, tag="segcnt", bufs=1)
                for b in range(NB):
                    st, sp = (b == 0), (b == NB - 1)
                    for n0, nsz in ((0, 512), (512, C - 512)):
                        nc.tensor.matmul(ps_cnt[:1, n0:n0 + nsz],
                                         lhsT=ones_col[:, :1],
                                         rhs=oh_tiles[b][:, n0:n0 + nsz],
                                         start=st, stop=sp)
                cnts_f = small.tile([1, 1024], bf16, tag="cnts_f")
                nc.vector.memset(cnts_f[:], 0.0)
                nc.vector.tensor_copy(cnts_f[:1, 0:C], ps_cnt[:1, 0:C])
                nc.sync.dma_start(out=cnts_dram[:, :], in_=cnts_f[:1, :])
                cnts_cols = small.tile([128, NCCH], bf16, tag="cnts_cols")
                nc.sync.dma_start(
                    out=cnts_cols[:, :],
                    in_=cnts_dram[0:1, :].rearrange("a (q p) -> (a p) q",
                                                    q=NCCH))
                nc.sync.dma_start(
                    out=redA_in[:, D:D + 1].rearrange("(q p) a -> p (q a)",
                                                      q=NCCH // 2),
                    in_=cnts_cols[:, 0:NCCH // 2])
                nc.sync.dma_start(
                    out=redB_in[:, D:D + 1].rearrange("(q p) a -> p (q a)",
                                                      q=NCCH // 2),
                    in_=cnts_cols[:, NCCH // 2:NCCH])

                # ---- phase 2b: segment sums (bf16 one-hot matmul) ----
                for ci, (c0, csz) in enumerate(c_chunks):
                    ps = psseg.tile([128, D], fp32, tag="segsum", bufs=3)
                    for b in range(NB):
                        for dh in (0, 512):
                            nc.tensor.matmul(
                                ps[:csz, dh:dh + 512],
                                lhsT=oh_tiles[b][:, c0:c0 + csz],
                                rhs=f_tiles[b][:, dh:dh + 512],
                                start=(b == 0), stop=(b == NB - 1))
                    sums_f = stream.tile([128, D], bf16, tag="sums_f",
                                         bufs=2)
                    if ci % 2 == 0:
                        nc.scalar.copy(sums_f[:csz, :], ps[:csz, :])
                    else:
                        nc.vector.tensor_copy(sums_f[:csz, :], ps[:csz, :])
                    if c0 < 512:
                        nc.sync.dma_start(out=redA_in[c0:c0 + csz, 0:D],
                                          in_=sums_f[:csz, :])
                    else:
                        nc.sync.dma_start(
                            out=redB_in[c0 - 512:c0 - 512 + csz, 0:D],
                            in_=sums_f[:csz, :])
                    if ci == 3:
                        nc.gpsimd.collective_compute(
                            "ReduceScatter", ALU.add,
                            ins=[redA_in.opt()], outs=[rs1_out.opt()],
                            replica_groups=[list(range(NCORES))])

            # ---- phase 3: ReduceScatter half 2 ----
            nc.gpsimd.collective_compute(
                "ReduceScatter", ALU.add,
                ins=[redB_in.opt()], outs=[rs2_out.opt()],
                replica_groups=[list(range(NCORES))])
            psmm = est.enter_context(
                tc.tile_pool(name="psmm", bufs=1, space="PSUM"))

            # ---- phase 4: momentum update of this core's 128 classes ----
            cen = stream.tile([128, D], fp32, tag="scrB")
            nc.sync.dma_start(out=cen[:, :], in_=centers_sh[:, :])
            rs_sb = stream.tile([128, RW], bf16, tag="rs_sb")
            csq_col = small.tile([128, 1], fp32, tag="csq_col")
            cn_bf = resid.tile([128, DBF], bf16, tag="cn_bf")
            for h, rsh in ((0, rs1_out), (1, rs2_out)):
                r0, r1 = h * 64, (h + 1) * 64
                nc.sync.dma_start(out=rs_sb[r0:r1, :], in_=rsh[:, :])
                cntc = small.tile([128, 1], fp32, tag="cntc")
                nc.vector.tensor_scalar_max(cntc[r0:r1],
                                            rs_sb[r0:r1, D:D + 1], 1.0)
                rcv = small.tile([128, 1], fp32, tag="rcv")
                nc.vector.reciprocal(rcv[r0:r1], cntc[r0:r1])
                w = small.tile([128, 1], fp32, tag="w")
                nc.vector.tensor_scalar(out=w[r0:r1],
                                        in0=rs_sb[r0:r1, D:D + 1],
                                        scalar1=0.0, scalar2=1.0 - MOM,
                                        op0=ALU.is_gt, op1=ALU.mult)
                m = small.tile([128, 1], fp32, tag="m")
                nc.vector.tensor_tensor(out=m[r0:r1], in0=w[r0:r1],
                                        in1=rcv[r0:r1], op=ALU.mult)
                u = small.tile([128, 1], fp32, tag="u")
                nc.vector.tensor_scalar(out=u[r0:r1], in0=w[r0:r1],
                                        scalar1=-1.0, scalar2=1.0,
                                        op0=ALU.mult, op1=ALU.add)
                t1 = stream.tile([128, D], fp32, tag="scrC")
                nc.scalar.mul(t1[r0:r1], cen[r0:r1, :], u[r0:r1, :1])
                cn_t = stream.tile([128, D], fp32, tag="cn_t")
                nc.vector.scalar_tensor_tensor(
                    out=cn_t[r0:r1, :], in0=rs_sb[r0:r1, 0:D],
                    scalar=m[r0:r1, :1], in1=t1[r0:r1, :],
                    op0=ALU.mult, op1=ALU.add)
                scr2 = stream.tile([128, D], bf16, tag="sqdump")
                nc.scalar.activation(out=scr2[r0:r1], in_=cn_t[r0:r1, :],
                                     func=AF.Square,
                                     accum_out=csq_col[r0:r1])
                nc.vector.tensor_copy(cn_bf[r0:r1, 0:D], cn_t[r0:r1, :])
                nc.vector.tensor_copy(cn_bf[r0:r1, D:D + 2].bitcast(fp32),
                                      csq_col[r0:r1, :])
                nc.vector.memset(cn_bf[r0:r1, D + 2:DBF], 0.0)
                nc.sync.dma_start(out=ag_in[r0:r1, :], in_=cn_bf[r0:r1, :])

            # local transposes of this core's CnT blocks -> ag2 payload
            stage = resid.tile([128, 1024], bf16, tag="stage")
            for dj in range(8):
                tpl = psmm.tile([128, 128], bf16, tag="tpl", bufs=2)
                nc.tensor.transpose(
                    out=tpl[:, :], in_=cn_bf[:, dj * 128:(dj + 1) * 128],
                    identity=ident_b[:, :])
                if dj % 2 == 0:
                    nc.scalar.copy(stage[:, dj * 128:(dj + 1) * 128],
                                   tpl[:, :])
                else:
                    nc.vector.tensor_copy(stage[:, dj * 128:(dj + 1) * 128],
                                          tpl[:, :])
            csq_bf = small.tile([1, 128], bf16, tag="csq_bf")
            tpc = psmm.tile([1, 128], fp32, tag="tpc", bufs=1)
            nc.tensor.transpose(out=tpc[:1, :], in_=csq_col[:, :1],
                                identity=ident_f[:, :])
            nc.vector.tensor_copy(csq_bf[:1, :], tpc[:1, :])
            nc.sync.dma_start(out=ag_in[128:256, 0:1024], in_=stage[:, :])
            nc.sync.dma_start(out=ag_in[256:257, 0:128], in_=csq_bf[:1, :])

            # ---- phase 5: one merged AllGather ----
            nc.gpsimd.collective_compute(
                "AllGather", ALU.bypass,
                ins=[ag_in.opt()], outs=[cn_dram.opt()],
                replica_groups=[list(range(NCORES))])

            # ---- phase 2c: features_adv norms (gpsimd; overlap the RS) ----
            xa_tiles, rina_tiles = [], []
            for b in range(NB):
                r0 = b * 128
                xa_t = stream.tile([128, D], fp32, tag="scrA")
                nc.sync.dma_start(out=xa_t[:], in_=feat_adv[r0:r0 + 128, :])
                ssa = small.tile([128, 1], fp32, tag="ss")
                scr = stream.tile([128, D], fp32, tag="scrB")
                nc.scalar.activation(out=scr[:], in_=xa_t[:],
                                     func=AF.Square, accum_out=ssa[:])
                nrma = small.tile([128, 1], fp32, tag="nrm")
                nc.scalar.activation(out=nrma[:], in_=ssa[:], func=AF.Sqrt)
                nc.vector.tensor_scalar_max(nrma[:], nrma[:], 1e-12)
                rina = resid.tile([128, 1], fp32, tag=f"rina{b}",
                                  name=f"rina{b}")
                nc.vector.reciprocal(rina[:], nrma[:])
                rina_tiles.append(rina)
                xa_bf = resid.tile([128, D], bf16, tag=f"xa{b}",
                                   name=f"xa{b}")
                nc.vector.tensor_scalar_mul(xa_bf[:], xa_t[:], rina[:, :1])
                xa_tiles.append(xa_bf)


            # pairwise operands: CnT tiles + csq row from the stage rows.
            # stage row d, col dj*128+j  ==  CnT[dj*128+d, local class j]
            stage_v = cn_dram[:, :].rearrange(
                "(k r) j -> r k j", k=NCORES)[128:256, :, 0:1024].rearrange(
                "p k (dj j) -> p k dj j", dj=8)
            cnt_sb = []
            for dj in range(8):
                ct = resid.tile([128, CPAD], bf16, tag=f"cnt{dj}",
                                name=f"cnt_sb{dj}")
                nc.sync.dma_start(out=ct[:], in_=stage_v[:, :, dj, :])
                cnt_sb.append(ct)
            csq_row = constp.tile([1, 1024], bf16, tag="csq_row")
            nc.sync.dma_start(
                out=csq_row[:],
                in_=cn_dram[:, :].rearrange("(k r) j -> r k j",
                                            k=NCORES)[256:257, :, 0:128])

            # ---- phase 6: intra losses via fused dots + gathered csq ----
            lg_tiles = []
            for b in range(NB):
                lg_t = resid.tile([128, 1], i32, tag=f"lg{b}",
                                  name=f"lg{b}")
                nc.sync.dma_start(out=lg_t[:], in_=labels_g[:, b:b + 1])
                lg_tiles.append(lg_t)

            rsel_t = small.tile([128, 1], i32, tag="rsel")
            nc.sync.dma_start(out=rsel_t[:], in_=rowsel[:, :])
            my_bf = resid.tile([128, DBF], bf16, tag="my_bf")
            nc.gpsimd.indirect_dma_start(
                out=my_bf[:], out_offset=None, in_=cn_dram[:, :],
                in_offset=bass.IndirectOffsetOnAxis(ap=rsel_t[:, :1], axis=0))


            # ---- phase 7: pairwise inter loss ----
            cmy_t = []
            for dj in range(8):
                tpm = psmm.tile([128, 128], bf16, tag="tpm", bufs=2)
                nc.tensor.transpose(
                    out=tpm[:, :], in_=my_bf[:, dj * 128:(dj + 1) * 128],
                    identity=ident_b[:, :])
                cm = resid.tile([128, 128], bf16, tag=f"cmy{dj}",
                                name=f"cmy{dj}")
                nc.scalar.mul(cm[:], tpm[:, :], -2.0)
                cmy_t.append(cm)

            # g_ps accumulates  -2 * Cn_my @ Cn.T  +  csq_row broadcast
            g_ps = psmm.tile([128, C], fp32, tag="gmm", bufs=1)
            for dj in range(8):
                for n0, nsz in ((0, 512), (512, C - 512)):
                    nc.tensor.matmul(g_ps[:, n0:n0 + nsz],
                                     lhsT=cmy_t[dj][:, :],
                                     rhs=cnt_sb[dj][:, n0:n0 + nsz],
                                     start=(dj == 0), stop=False)
            for n0, nsz in ((0, 512), (512, C - 512)):
                nc.tensor.matmul(g_ps[:, n0:n0 + nsz], lhsT=ones_row[:1, :],
                                 rhs=csq_row[:1, n0:n0 + nsz],
                                 start=False, stop=True)
            d2b = stream.tile([128, C], fp32, tag="scrB")
            nc.vector.tensor_scalar(
                out=d2b[:], in0=g_ps[:, :],
                scalar1=my_bf[:, D:D + 2].bitcast(fp32)[:, :1],
                scalar2=0.0, op0=ALU.add, op1=ALU.max)
            dst = stream.tile([128, C], fp32, tag="scrC")
            nc.scalar.activation(out=dst[:], in_=d2b[:], func=AF.Sqrt)
            term = stream.tile([128, C], fp32, tag="scrA")
            inter_rows = small.tile([128, 1], fp32, tag="inter_rows")
            nc.scalar.activation(out=term[:], in_=dst[:],
                                 func=AF.Relu, bias=1.0, scale=-1.0,
                                 accum_out=inter_rows[:])


            dots_f = resid.tile([128, NB], fp32, tag="dots_f")
            gq = resid.tile([128, NB], fp32, tag="gq")
            ssa_t = small.tile([128, NB], fp32, tag="ssa_t")
            for b in range(NB):
                g_t = stream.tile([128, DBF], bf16, tag="gat", bufs=4)
                nc.gpsimd.indirect_dma_start(
                    out=g_t[:], out_offset=None, in_=cn_dram[:, :],
                    in_offset=bass.IndirectOffsetOnAxis(
                        ap=lg_tiles[b][:, :1], axis=0))
                nc.vector.tensor_copy(gq[:, b:b + 1],
                                      g_t[:, D:D + 2].bitcast(fp32))
                prodf = stream.tile([128, D], bf16, tag="pdump", bufs=3)
                nc.vector.scalar_tensor_tensor(
                    out=prodf[:], in0=f_tiles[b][:], scalar=1.0,
                    in1=g_t[:, 0:D], op0=ALU.mult, op1=ALU.mult,
                    accum_out=dots_f[:, b:b + 1])
                da_t = stream.tile([128, D], bf16, tag="pdump", bufs=3)
                nc.vector.tensor_tensor(out=da_t[:], in0=xa_tiles[b][:],
                                        in1=g_t[:, 0:D],
                                        op=ALU.subtract)
                scra = stream.tile([128, D], bf16, tag="sqd2", bufs=3)
                nc.scalar.activation(out=scra[:], in_=da_t[:],
                                     func=AF.Square,
                                     accum_out=ssa_t[:, b:b + 1])
            # ssf = 1 + gq - 2*dots_f
            gq1 = small.tile([128, NB], fp32, tag="gq1")
            nc.vector.tensor_scalar(out=gq1[:], in0=gq[:], scalar1=1.0,
                                    scalar2=None, op0=ALU.add)
            ssf_t = small.tile([128, NB], fp32, tag="ssf_t")
            nc.vector.scalar_tensor_tensor(
                out=ssf_t[:], in0=dots_f[:], scalar=-2.0, in1=gq1[:],
                op0=ALU.mult, op1=ALU.add)
            nc.vector.tensor_scalar_max(ssf_t[:], ssf_t[:], 0.0)

            dist_f = small.tile([128, NB], fp32, tag="dist_f")
            nc.scalar.activation(out=dist_f[:], in_=ssf_t[:], func=AF.Sqrt)
            dist_a = small.tile([128, NB], fp32, tag="dist_a")
            nc.scalar.activation(out=dist_a[:], in_=ssa_t[:], func=AF.Sqrt)
            ir_f = small.tile([128, 1], fp32, tag="ir_f")
            nc.vector.tensor_reduce(out=ir_f[:], in_=dist_f[:], axis=AX.X,
                                    op=ALU.add)
            ir_a = small.tile([128, 1], fp32, tag="ir_a")
            nc.vector.tensor_reduce(out=ir_a[:], in_=dist_a[:], axis=AX.X,
                                    op=ALU.add)
            intra_rows = small.tile([128, 1], fp32, tag="intra_rows")
            nc.vector.tensor_add(intra_rows[:], ir_f[:], ir_a[:])

            # ---- phase 8: final reduce + tiny AllReduce + formula ----
            partials = small.tile([128, 2], fp32, tag="partials")
            nc.vector.memset(partials[:], 0.0)
            nc.vector.tensor_copy(partials[:, 0:1], intra_rows[:])
            nc.vector.tensor_copy(partials[:CROWS, 1:2],
                                  inter_rows[:CROWS, :])
            pr = small.tile([1, 2], fp32, tag="pr")
            nc.gpsimd.tensor_reduce(out=pr[:1, :], in_=partials[:, :],
                                    axis=AX.C, op=ALU.add)
            nc.sync.dma_start(out=out[0:1, 0:2], in_=pr[:1, :])

    nc.compile()
    return nc


def _get_nc():
    if "nc" not in _state:
        _state["nc"] = _build()
    return _state["nc"]


def kernel(features, features_adv, centers, labels):
    from concourse import bass_utils

    nc = _get_nc()
    import ml_dtypes
    features_bf = np.ascontiguousarray(
        np.asarray(features, dtype=np.float32).astype(ml_dtypes.bfloat16))
    features_adv = np.ascontiguousarray(np.asarray(features_adv,
                                                   dtype=np.float32))
    centers_np = np.asarray(centers, dtype=np.float32)
    centers_pad = np.zeros((CPAD, D), dtype=np.float32)
    centers_pad[:C] = centers_np
    labels_i32 = np.ascontiguousarray(
        np.asarray(labels).astype(np.int32).reshape(B, 1))
    # gather-row remap for the split-RS shard layout: core k owns classes
    # [64k, 64k+64) (slot rows 0:64) and [512+64k, 512+64k+64) (rows 64:128)
    def _rowpos(c):
        c = np.asarray(c, dtype=np.int64)
        lo = AGR * (c // 64) + (c % 64)
        hi = AGR * ((c - 512) // 64) + 64 + ((c - 512) % 64)
        return np.where(c < 512, lo, hi)

    labels_gr = _rowpos(labels_i32).astype(np.int32).reshape(B)

    in_maps = []
    for k in range(NCORES):
        sl = slice(k * BLOC, (k + 1) * BLOC)
        rsel_c = np.zeros((128,), dtype=np.int64)
        rsel_c[:CROWS] = np.arange(k * CROWS, (k + 1) * CROWS)
        rsel = _rowpos(rsel_c).astype(np.int32).reshape(128, 1)
        in_maps.append({
            "features": features_bf[sl],
            "features_adv": features_adv[sl],
            "centers_sh": np.ascontiguousarray(np.concatenate([
                centers_pad[k * 64:(k + 1) * 64],
                centers_pad[512 + k * 64:512 + (k + 1) * 64]])),
            "labels": labels_i32[sl],
            "labels_g": np.ascontiguousarray(
                labels_gr[sl].reshape(NB, 128).T),
            "rowsel": rsel,
        })

    res = bass_utils.run_bass_kernel_spmd(
        nc, in_maps, core_ids=list(range(NCORES)),
        trace=bool(int(os.environ.get("AFD_TRACE", "0"))))
    _state["last_results"] = res
    parts = np.stack([res.results[k]["out"][0] for k in range(NCORES)])
    intra_sum = float(parts[:, 0].sum())
    inter_sum = float(parts[:, 1].sum())
    val = intra_sum / B - 0.25 * (inter_sum - C) / N_PAIRS
    return np.asarray(np.float32(val))


# revision 32
# speedup vs baseline: 1.0504x; 1.0504x over previous
"""AFD loss kernel for 8 TRN2 NeuronCores (Bass/Tile).

Algorithm (matches the reference loss_fn):
  f  = l2norm(features); fa = l2norm(features_adv)
  per-class sums/counts of f via one-hot matmul   (batch-sharded)
  centers_new = where(counts>0, 0.9*centers + 0.1*sums/max(counts,1), centers)
  intra = mean ||f - centers_new[labels]|| + mean ||fa - centers_new[labels]||
        with ||f - c||^2 = ||f||^2 - 2 f.c + ||c||^2   (fused dot + gathered csq)
  inter = sum_{i<j} relu(1 - ||ci - cj||) / n_pairs   (symmetric full-sum trick)
  loss  = intra - 0.5 * inter

v4 structure:
  - batch-sharded inputs; centers row-sharded on host (128 rows/core)
  - segment sums via bf16 one-hot matmuls into fp32 PSUM
  - ReduceScatter (fp32; rows 128k..128k+128 land on core k; counts in col
    1024) -> per-core momentum update of its own 128 classes
  - AllGather #1: updated center rows (bf16 + exact fp32 csq bitcast cols)
    -> full centers in DRAM for label/row gathers
  - AllGather #2: locally PE-transposed CnT blocks + bf16 csq row -> pairwise
    matmul operands with no post-AG transposes
  - intra via fused scalar_tensor_tensor dot products; inter via one row-
    sharded matmul; final tiny AllReduce combines the partial scalars
"""

import os
from contextlib import ExitStack

import numpy as np

NCORES = 8
B = 8192
D = 1024
C = 1000
BLOC = B // NCORES          # 1024 rows per core
NB = BLOC // 128            # 8 batch tiles per core
CROWS = C // NCORES         # 125 pairwise rows per core
MOM = 0.9
N_PAIRS = C * (C - 1) / 2.0
NCCH = (C + 127) // 128     # 8 class chunks
CPAD = 1024                 # classes padded to full chunks
DBF = D + 16                # bf16 center row: D data + csq(f32 as 2 bf16) + pad
RW = D + 1                  # reduce row width: sums + count column
AGR = 257                   # AG rows/rank: 128 cn + 128 cnT-stage + 1 csq

_state = {}


def _build():
    import concourse.bacc as bacc
    import concourse.bass as bass
    import concourse.mybir as mybir
    import concourse.tile as tile
    from concourse.masks import make_identity

    fp32 = mybir.dt.float32
    bf16 = mybir.dt.bfloat16
    i32 = mybir.dt.int32
    AF = mybir.ActivationFunctionType
    ALU = mybir.AluOpType
    AX = mybir.AxisListType

    nc = bacc.Bacc("TRN2", target_bir_lowering=False, debug=False,
                   num_devices=NCORES)

    feat = nc.dram_tensor("features", [BLOC, D], bf16, kind="ExternalInput")
    feat_adv = nc.dram_tensor("features_adv", [BLOC, D], fp32,
                              kind="ExternalInput")
    centers_sh = nc.dram_tensor("centers_sh", [128, D], fp32,
                                kind="ExternalInput")
    labels = nc.dram_tensor("labels", [BLOC, 1], i32, kind="ExternalInput")
    labels_g = nc.dram_tensor("labels_g", [128, NB], i32,
                              kind="ExternalInput")
    out = nc.dram_tensor("out", [1, 2], fp32, kind="ExternalOutput")

    c_chunks = [(i * 128, min(128, C - i * 128)) for i in range(NCCH)]

    with tile.TileContext(nc) as tc:
        with (
            tc.tile_pool(name="const", bufs=1) as constp,
            tc.tile_pool(name="resid", bufs=1) as resid,
            tc.tile_pool(name="stream", bufs=2) as stream,
            tc.tile_pool(name="small", bufs=4) as small,
            tc.tile_pool(name="dram", bufs=1, space="DRAM") as dram,
            ExitStack() as est,
        ):
            # ---- constants ----
            iota_t = constp.tile([128, C], fp32, tag="iota")
            nc.gpsimd.iota(iota_t[:], pattern=[[1, C]], base=0,
                           channel_multiplier=0,
                           allow_small_or_imprecise_dtypes=True)
            ones_col = constp.tile([128, 1], bf16, tag="ones_col")
            nc.vector.memset(ones_col[:], 1.0)
            ones_row = constp.tile([1, 128], bf16, tag="ones_row")
            nc.vector.memset(ones_row[:], 1.0)
            ident_f = constp.tile([128, 128], fp32, tag="ident_f")
            make_identity(nc, ident_f[:])
            ident_b = constp.tile([128, 128], bf16, tag="ident_b")
            make_identity(nc, ident_b[:])

            # DRAM bounces
            red_in = dram.tile([CPAD, RW], bf16, tag="red_in")
            rs_out = dram.tile([128, RW], bf16, tag="rs_out")
            ag_in = dram.tile([AGR, DBF], bf16, tag="ag_in")
            cn_dram = dram.tile([AGR * NCORES, DBF], bf16, tag="cn",
                                addr_space="Shared")
            cnts_dram = dram.tile([1, 1024], bf16, tag="cnts_dram")

            # zero the padded class rows of red_in once (classes 1000..1023)
            zero_t = constp.tile([128, RW], bf16, tag="zero_t")
            nc.vector.memset(zero_t[:], 0.0)
            nc.sync.dma_start(out=red_in[C:CPAD, :], in_=zero_t[:CPAD - C, :])

            f_tiles, lab_tiles = [], []

            with (tc.tile_pool(name="ohp", bufs=1) as ohp,
                  tc.tile_pool(name="psseg", bufs=1, space="PSUM") as psseg):
                # ---- phase 1: load + normalize f (bf16); one-hot (bf16) ----
                oh_tiles, x_tiles = [], []
                for b in range(NB):
                    r0 = b * 128
                    x_t = stream.tile([128, D], bf16, tag="xin", bufs=3,
                                      name=f"x{b}")
                    nc.sync.dma_start(out=x_t[:], in_=feat[r0:r0 + 128, :])
                    x_tiles.append(x_t)
                    lab_t = resid.tile([128, 1], i32, tag=f"lab{b}",
                                       name=f"lab{b}")
                    nc.sync.dma_start(out=lab_t[:], in_=labels[r0:r0 + 128, :])
                    lab_tiles.append(lab_t)
                for b in range(NB):
                    x_t = x_tiles[b]
                    ss = small.tile([128, 1], fp32, tag="ss")
                    scr = stream.tile([128, D], fp32, tag="scrB")
                    nc.scalar.activation(out=scr[:], in_=x_t[:],
                                         func=AF.Square, accum_out=ss[:])
                    nrm = small.tile([128, 1], fp32, tag="nrm")
                    nc.scalar.activation(out=nrm[:], in_=ss[:], func=AF.Sqrt)
                    nc.vector.tensor_scalar_max(nrm[:], nrm[:], 1e-12)
                    rin = small.tile([128, 1], fp32, tag="rin")
                    nc.vector.reciprocal(rin[:], nrm[:])
                    f_t = resid.tile([128, D], bf16, tag=f"f{b}",
                                     name=f"f{b}")
                    nc.vector.tensor_scalar_mul(f_t[:], x_t[:], rin[:, :1])
                    f_tiles.append(f_t)
                    lab_f = small.tile([128, 1], fp32, tag="labf")
                    nc.vector.tensor_copy(lab_f[:], lab_tiles[b][:])
                    oh_t = ohp.tile([128, C], bf16, tag=f"oh{b}",
                                    name=f"oh{b}")
                    nc.vector.tensor_scalar(
                        out=oh_t[:], in0=iota_t[:], scalar1=lab_f[:, :1],
                        scalar2=None, op0=ALU.is_equal)
                    oh_tiles.append(oh_t)

                # ---- phase 2a: counts row -> column 1024 of red_in ----
                ps_cnt = psseg.tile([1, 1024], fp32, tag="segcnt", bufs=1)
                for b in range(NB):
                    st, sp = (b == 0), (b == NB - 1)
                    for n0, nsz in ((0, 512), (512, C - 512)):
                        nc.tensor.matmul(ps_cnt[:1, n0:n0 + nsz],
                                         lhsT=ones_col[:, :1],
                                         rhs=oh_tiles[b][:, n0:n0 + nsz],
                                         start=st, stop=sp)
                cnts_f = small.tile([1, 1024], bf16, tag="cnts_f")
                nc.vector.memset(cnts_f[:], 0.0)
                nc.vector.tensor_copy(cnts_f[:1, 0:C], ps_cnt[:1, 0:C])
                nc.sync.dma_start(out=cnts_dram[:, :], in_=cnts_f[:1, :])
                cnts_cols = small.tile([128, NCCH], bf16, tag="cnts_cols")
                nc.sync.dma_start(
                    out=cnts_cols[:, :],
                    in_=cnts_dram[0:1, :].rearrange("a (q p) -> (a p) q",
                                                    q=NCCH))
                nc.sync.dma_start(
                    out=red_in[:, D:D + 1].rearrange("(q p) a -> p (q a)",
                                                     q=NCCH),
                    in_=cnts_cols[:, :])

                # ---- phase 2b: segment sums (bf16 one-hot matmul) ----
                for ci, (c0, csz) in enumerate(c_chunks):
                    ps = psseg.tile([128, D], fp32, tag="segsum", bufs=3)
                    for b in range(NB):
                        for dh in (0, 512):
                            nc.tensor.matmul(
                                ps[:csz, dh:dh + 512],
                                lhsT=oh_tiles[b][:, c0:c0 + csz],
                                rhs=f_tiles[b][:, dh:dh + 512],
                                start=(b == 0), stop=(b == NB - 1))
                    sums_f = stream.tile([128, D], bf16, tag="sums_f",
                                         bufs=2)
                    if ci % 2 == 0:
                        nc.scalar.copy(sums_f[:csz, :], ps[:csz, :])
                    else:
                        nc.vector.tensor_copy(sums_f[:csz, :], ps[:csz, :])
                    nc.sync.dma_start(out=red_in[c0:c0 + csz, 0:D],
                                      in_=sums_f[:csz, :])

            # ---- phase 3: ReduceScatter (core k owns classes 128k..) ----
            nc.gpsimd.collective_compute(
                "ReduceScatter", ALU.add,
                ins=[red_in.opt()], outs=[rs_out.opt()],
                replica_groups=[list(range(NCORES))])
            psmm = est.enter_context(
                tc.tile_pool(name="psmm", bufs=1, space="PSUM"))

            # ---- phase 4: momentum update of this core's 128 classes ----
            cen = stream.tile([128, D], fp32, tag="scrB")
            nc.sync.dma_start(out=cen[:, :], in_=centers_sh[:, :])
            rs_sb = stream.tile([128, RW], bf16, tag="rs_sb")
            nc.sync.dma_start(out=rs_sb[:, :], in_=rs_out[:, :])
            csq_col = small.tile([128, 1], fp32, tag="csq_col")
            cn_bf = resid.tile([128, DBF], bf16, tag="cn_bf")
            cntc = small.tile([128, 1], fp32, tag="cntc")
            nc.vector.tensor_scalar_max(cntc[:], rs_sb[:, D:D + 1], 1.0)
            rcv = small.tile([128, 1], fp32, tag="rcv")
            nc.vector.reciprocal(rcv[:], cntc[:])
            w = small.tile([128, 1], fp32, tag="w")
            nc.vector.tensor_scalar(out=w[:], in0=rs_sb[:, D:D + 1],
                                    scalar1=0.0, scalar2=1.0 - MOM,
                                    op0=ALU.is_gt, op1=ALU.mult)
            m = small.tile([128, 1], fp32, tag="m")
            nc.vector.tensor_tensor(out=m[:], in0=w[:], in1=rcv[:],
                                    op=ALU.mult)
            u = small.tile([128, 1], fp32, tag="u")
            nc.vector.tensor_scalar(out=u[:], in0=w[:], scalar1=-1.0,
                                    scalar2=1.0, op0=ALU.mult, op1=ALU.add)
            t1 = stream.tile([128, D], fp32, tag="scrC")
            nc.scalar.mul(t1[:], cen[:, :], u[:, :1])
            cn_t = stream.tile([128, D], fp32, tag="cn_t")
            nc.vector.scalar_tensor_tensor(
                out=cn_t[:, :], in0=rs_sb[:, 0:D], scalar=m[:, :1],
                in1=t1[:, :], op0=ALU.mult, op1=ALU.add)
            scr2 = stream.tile([128, D], bf16, tag="sqdump")
            nc.scalar.activation(out=scr2[:], in_=cn_t[:, :],
                                 func=AF.Square, accum_out=csq_col[:])
            nc.vector.tensor_copy(cn_bf[:, 0:D], cn_t[:, :])
            nc.vector.tensor_copy(cn_bf[:, D:D + 2].bitcast(fp32),
                                  csq_col[:, :])
            nc.vector.memset(cn_bf[:, D + 2:DBF], 0.0)
            nc.sync.dma_start(out=ag_in[0:128, :], in_=cn_bf[:, :])

            # local transposes of this core's CnT blocks -> ag2 payload
            stage = resid.tile([128, 1024], bf16, tag="stage")
            for dj in range(8):
                tpl = psmm.tile([128, 128], bf16, tag="tpl", bufs=2)
                nc.tensor.transpose(
                    out=tpl[:, :], in_=cn_bf[:, dj * 128:(dj + 1) * 128],
                    identity=ident_b[:, :])
                if dj % 2 == 0:
                    nc.scalar.copy(stage[:, dj * 128:(dj + 1) * 128],
                                   tpl[:, :])
                else:
                    nc.vector.tensor_copy(stage[:, dj * 128:(dj + 1) * 128],
                                          tpl[:, :])
            myT = resid.tile([128, 1024], bf16, tag="myT")
            nc.vector.tensor_scalar_mul(myT[:], stage[:, :], -2.0)
            csq_bf = small.tile([1, 128], bf16, tag="csq_bf")
            tpc = psmm.tile([1, 128], fp32, tag="tpc", bufs=1)
            nc.tensor.transpose(out=tpc[:1, :], in_=csq_col[:, :1],
                                identity=ident_f[:, :])
            nc.vector.tensor_copy(csq_bf[:1, :], tpc[:1, :])
            nc.sync.dma_start(out=ag_in[128:256, 0:1024], in_=stage[:, :])
            nc.sync.dma_start(out=ag_in[256:257, 0:128], in_=csq_bf[:1, :])

            # ---- phase 5: one merged AllGather ----
            nc.gpsimd.collective_compute(
                "AllGather", ALU.bypass,
                ins=[ag_in.opt()], outs=[cn_dram.opt()],
                replica_groups=[list(range(NCORES))])

            # ---- phase 2c: features_adv norms (gpsimd; overlap the RS) ----
            xa_tiles, rina_tiles = [], []
            for b in range(NB):
                r0 = b * 128
                xa_t = stream.tile([128, D], fp32, tag="scrA")
                nc.sync.dma_start(out=xa_t[:], in_=feat_adv[r0:r0 + 128, :])
                ssa = small.tile([128, 1], fp32, tag="ss")
                scr = stream.tile([128, D], fp32, tag="scrB")
                nc.scalar.activation(out=scr[:], in_=xa_t[:],
                                     func=AF.Square, accum_out=ssa[:])
                nrma = small.tile([128, 1], fp32, tag="nrm")
                nc.scalar.activation(out=nrma[:], in_=ssa[:], func=AF.Sqrt)
                nc.vector.tensor_scalar_max(nrma[:], nrma[:], 1e-12)
                rina = resid.tile([128, 1], fp32, tag=f"rina{b}",
                                  name=f"rina{b}")
                nc.vector.reciprocal(rina[:], nrma[:])
                rina_tiles.append(rina)
                xa_bf = resid.tile([128, D], bf16, tag=f"xa{b}",
                                   name=f"xa{b}")
                nc.vector.tensor_scalar_mul(xa_bf[:], xa_t[:], rina[:, :1])
                xa_tiles.append(xa_bf)


            # pairwise operands: CnT tiles + csq row from the stage rows.
            # stage row d, col dj*128+j  ==  CnT[dj*128+d, local class j]
            stage_v = cn_dram[:, :].rearrange(
                "(k r) j -> r k j", k=NCORES)[128:256, :, 0:1024].rearrange(
                "p k (dj j) -> p k dj j", dj=8)
            cnt_sb = []
            for dj in range(8):
                ct = resid.tile([128, CPAD], bf16, tag=f"cnt{dj}",
                                name=f"cnt_sb{dj}")
                eng = nc.sync if dj % 2 == 0 else nc.scalar
                eng.dma_start(out=ct[:], in_=stage_v[:, :, dj, :])
                cnt_sb.append(ct)
            csq_row = constp.tile([1, 1024], bf16, tag="csq_row")
            nc.sync.dma_start(
                out=csq_row[:],
                in_=cn_dram[:, :].rearrange("(k r) j -> r k j",
                                            k=NCORES)[256:257, :, 0:128])

            # ---- phase 6: intra losses via fused dots + gathered csq ----
            lg_tiles = []
            for b in range(NB):
                lg_t = resid.tile([128, 1], i32, tag=f"lg{b}",
                                  name=f"lg{b}")
                nc.sync.dma_start(out=lg_t[:], in_=labels_g[:, b:b + 1])
                lg_tiles.append(lg_t)

            # ---- phase 7: pairwise inter loss (rows = this core's own
            # 128 classes; zero pad rows contribute exactly 0) ----
            g_ps = psmm.tile([128, C], fp32, tag="gmm", bufs=1)
            for dj in range(8):
                for n0, nsz in ((0, 512), (512, C - 512)):
                    nc.tensor.matmul(g_ps[:, n0:n0 + nsz],
                                     lhsT=myT[:, dj * 128:(dj + 1) * 128],
                                     rhs=cnt_sb[dj][:, n0:n0 + nsz],
                                     start=(dj == 0), stop=False)
            for n0, nsz in ((0, 512), (512, C - 512)):
                nc.tensor.matmul(g_ps[:, n0:n0 + nsz], lhsT=ones_row[:1, :],
                                 rhs=csq_row[:1, n0:n0 + nsz],
                                 start=False, stop=True)
            d2b = stream.tile([128, C], fp32, tag="scrB")
            nc.vector.tensor_scalar(
                out=d2b[:], in0=g_ps[:, :],
                scalar1=csq_col[:, :1],
                scalar2=0.0, op0=ALU.add, op1=ALU.max)
            dst = stream.tile([128, C], fp32, tag="scrC")
            nc.scalar.activation(out=dst[:], in_=d2b[:], func=AF.Sqrt)
            term = stream.tile([128, C], fp32, tag="scrA")
            inter_rows = small.tile([128, 1], fp32, tag="inter_rows")
            nc.scalar.activation(out=term[:], in_=dst[:],
                                 func=AF.Relu, bias=1.0, scale=-1.0,
                                 accum_out=inter_rows[:])


            dots_f = resid.tile([128, NB], fp32, tag="dots_f")
            gq = resid.tile([128, NB], fp32, tag="gq")
            ssa_t = small.tile([128, NB], fp32, tag="ssa_t")
            for b in range(NB):
                g_t = stream.tile([128, DBF], bf16, tag="gat", bufs=4)
                nc.gpsimd.indirect_dma_start(
                    out=g_t[:], out_offset=None, in_=cn_dram[:, :],
                    in_offset=bass.IndirectOffsetOnAxis(
                        ap=lg_tiles[b][:, :1], axis=0))
                nc.vector.tensor_copy(gq[:, b:b + 1],
                                      g_t[:, D:D + 2].bitcast(fp32))
                prodf = stream.tile([128, D], bf16, tag="pdump", bufs=3)
                nc.vector.scalar_tensor_tensor(
                    out=prodf[:], in0=f_tiles[b][:], scalar=1.0,
                    in1=g_t[:, 0:D], op0=ALU.mult, op1=ALU.mult,
                    accum_out=dots_f[:, b:b + 1])
                da_t = stream.tile([128, D], bf16, tag="pdump", bufs=3)
                nc.vector.tensor_tensor(out=da_t[:], in0=xa_tiles[b][:],
                                        in1=g_t[:, 0:D],
                                        op=ALU.subtract)
                scra = stream.tile([128, D], bf16, tag="sqd2", bufs=3)
                nc.scalar.activation(out=scra[:], in_=da_t[:],
                                     func=AF.Square,
                                     accum_out=ssa_t[:, b:b + 1])
            # ssf = 1 + gq - 2*dots_f
            gq1 = small.tile([128, NB], fp32, tag="gq1")
            nc.vector.tensor_scalar(out=gq1[:], in0=gq[:], scalar1=1.0,
                                    scalar2=None, op0=ALU.add)
            ssf_t = small.tile([128, NB], fp32, tag="ssf_t")
            nc.vector.scalar_tensor_tensor(
                out=ssf_t[:], in0=dots_f[:], scalar=-2.0, in1=gq1[:],
                op0=ALU.mult, op1=ALU.add)
            nc.vector.tensor_scalar_max(ssf_t[:], ssf_t[:], 0.0)

            dist_f = small.tile([128, NB], fp32, tag="dist_f")
            nc.scalar.activation(out=dist_f[:], in_=ssf_t[:], func=AF.Sqrt)
            dist_a = small.tile([128, NB], fp32, tag="dist_a")
            nc.scalar.activation(out=dist_a[:], in_=ssa_t[:], func=AF.Sqrt)
            ir_f = small.tile([128, 1], fp32, tag="ir_f")
            nc.vector.tensor_reduce(out=ir_f[:], in_=dist_f[:], axis=AX.X,
                                    op=ALU.add)
            ir_a = small.tile([128, 1], fp32, tag="ir_a")
            nc.vector.tensor_reduce(out=ir_a[:], in_=dist_a[:], axis=AX.X,
                                    op=ALU.add)
            intra_rows = small.tile([128, 1], fp32, tag="intra_rows")
            nc.vector.tensor_add(intra_rows[:], ir_f[:], ir_a[:])

            # ---- phase 8: final reduce + tiny AllReduce + formula ----
            partials = small.tile([128, 2], fp32, tag="partials")
            nc.vector.memset(partials[:], 0.0)
            nc.vector.tensor_copy(partials[:, 0:1], intra_rows[:])
            nc.vector.tensor_copy(partials[:, 1:2], inter_rows[:, :])
            pr = small.tile([1, 2], fp32, tag="pr")
            nc.gpsimd.tensor_reduce(out=pr[:1, :], in_=partials[:, :],
                                    axis=AX.C, op=ALU.add)
            nc.sync.dma_start(out=out[0:1, 0:2], in_=pr[:1, :])

    nc.compile()
    return nc


def _get_nc():
    if "nc" not in _state:
        _state["nc"] = _build()
    return _state["nc"]


def kernel(features, features_adv, centers, labels):
    from concourse import bass_utils

    nc = _get_nc()
    import ml_dtypes
    features_bf = np.ascontiguousarray(
        np.asarray(features, dtype=np.float32).astype(ml_dtypes.bfloat16))
    features_adv = np.ascontiguousarray(np.asarray(features_adv,
                                                   dtype=np.float32))
    centers_np = np.asarray(centers, dtype=np.float32)
    centers_pad = np.zeros((CPAD, D), dtype=np.float32)
    centers_pad[:C] = centers_np
    labels_i32 = np.ascontiguousarray(
        np.asarray(labels).astype(np.int32).reshape(B, 1))
    # gather-row remap: class c lives at AG row 257*(c//128) + c%128
    labels_gr = (AGR * (labels_i32 >> 7) + (labels_i32 & 127)).astype(
        np.int32).reshape(B)

    in_maps = []
    for k in range(NCORES):
        sl = slice(k * BLOC, (k + 1) * BLOC)

        in_maps.append({
            "features": features_bf[sl],
            "features_adv": features_adv[sl],
            "centers_sh": np.ascontiguousarray(
                centers_pad[k * 128:(k + 1) * 128]),
            "labels": labels_i32[sl],
            "labels_g": np.ascontiguousarray(
                labels_gr[sl].reshape(NB, 128).T),
        })

    res = bass_utils.run_bass_kernel_spmd(
        nc, in_maps, core_ids=list(range(NCORES)),
        trace=bool(int(os.environ.get("AFD_TRACE", "0"))))
    _state["last_results"] = res
    parts = np.stack([res.results[k]["out"][0] for k in range(NCORES)])
    intra_sum = float(parts[:, 0].sum())
    inter_sum = float(parts[:, 1].sum())
    val = intra_sum / B - 0.25 * (inter_sum - C) / N_PAIRS
    return np.asarray(np.float32(val))


# revision 33
# speedup vs baseline: 1.1443x; 1.0894x over previous
"""AFD loss kernel for 8 TRN2 NeuronCores (Bass/Tile).

Algorithm (matches the reference loss_fn):
  f  = l2norm(features); fa = l2norm(features_adv)
  per-class sums/counts of f via one-hot matmul   (batch-sharded)
  centers_new = where(counts>0, 0.9*centers + 0.1*sums/max(counts,1), centers)
  intra = mean ||f - centers_new[labels]|| + mean ||fa - centers_new[labels]||
        with ||f - c||^2 = ||f||^2 - 2 f.c + ||c||^2   (fused dot + gathered csq)
  inter = sum_{i<j} relu(1 - ||ci - cj||) / n_pairs   (symmetric full-sum trick)
  loss  = intra - 0.5 * inter

v4 structure:
  - batch-sharded inputs; centers row-sharded on host (128 rows/core)
  - segment sums via bf16 one-hot matmuls into fp32 PSUM
  - ReduceScatter (fp32; rows 128k..128k+128 land on core k; counts in col
    1024) -> per-core momentum update of its own 128 classes
  - AllGather #1: updated center rows (bf16 + exact fp32 csq bitcast cols)
    -> full centers in DRAM for label/row gathers
  - AllGather #2: locally PE-transposed CnT blocks + bf16 csq row -> pairwise
    matmul operands with no post-AG transposes
  - intra via fused scalar_tensor_tensor dot products; inter via one row-
    sharded matmul; final tiny AllReduce combines the partial scalars
"""

import os
from contextlib import ExitStack

import numpy as np

NCORES = 8
B = 8192
D = 1024
C = 1000
BLOC = B // NCORES          # 1024 rows per core
NB = BLOC // 128            # 8 batch tiles per core
CROWS = C // NCORES         # 125 pairwise rows per core
MOM = 0.9
N_PAIRS = C * (C - 1) / 2.0
NCCH = (C + 127) // 128     # 8 class chunks
CPAD = 1024                 # classes padded to full chunks
DBF = D + 16                # bf16 center row: D data + csq(f32 as 2 bf16) + pad
RW = D + 1                  # reduce row width: sums + count column
AGR = 257                   # AG rows/rank: 128 cn + 128 cnT-stage + 1 csq

_state = {}


def _build():
    import concourse.bacc as bacc
    import concourse.bass as bass
    import concourse.mybir as mybir
    import concourse.tile as tile
    from concourse.masks import make_identity

    fp32 = mybir.dt.float32
    bf16 = mybir.dt.bfloat16
    i32 = mybir.dt.int32
    AF = mybir.ActivationFunctionType
    ALU = mybir.AluOpType
    AX = mybir.AxisListType

    nc = bacc.Bacc("TRN2", target_bir_lowering=False, debug=False,
                   num_devices=NCORES)

    feat = nc.dram_tensor("features", [BLOC, D], bf16, kind="ExternalInput")
    feat_adv = nc.dram_tensor("features_adv", [BLOC, D], fp32,
                              kind="ExternalInput")
    centers_sh = nc.dram_tensor("centers_sh", [128, D], fp32,
                                kind="ExternalInput")
    labels = nc.dram_tensor("labels", [BLOC, 1], i32, kind="ExternalInput")
    labels_g = nc.dram_tensor("labels_g", [128, NB], i32,
                              kind="ExternalInput")
    out = nc.dram_tensor("out", [1, 2], fp32, kind="ExternalOutput")

    c_chunks = [(i * 128, min(128, C - i * 128)) for i in range(NCCH)]

    with tile.TileContext(nc) as tc:
        with (
            tc.tile_pool(name="const", bufs=1) as constp,
            tc.tile_pool(name="resid", bufs=1) as resid,
            tc.tile_pool(name="stream", bufs=2) as stream,
            tc.tile_pool(name="small", bufs=4) as small,
            tc.tile_pool(name="dram", bufs=1, space="DRAM") as dram,
            ExitStack() as est,
        ):
            # ---- constants ----
            iota_t = constp.tile([128, C], fp32, tag="iota")
            nc.gpsimd.iota(iota_t[:], pattern=[[1, C]], base=0,
                           channel_multiplier=0,
                           allow_small_or_imprecise_dtypes=True)
            ones_col = constp.tile([128, 1], bf16, tag="ones_col")
            nc.vector.memset(ones_col[:], 1.0)
            ones_row = constp.tile([1, 128], bf16, tag="ones_row")
            nc.vector.memset(ones_row[:], 1.0)
            ident_f = constp.tile([128, 128], fp32, tag="ident_f")
            make_identity(nc, ident_f[:])
            ident_b = constp.tile([128, 128], bf16, tag="ident_b")
            make_identity(nc, ident_b[:])

            # DRAM bounces
            red_in = dram.tile([CPAD, RW], bf16, tag="red_in")
            rs_out = dram.tile([128, RW], bf16, tag="rs_out")
            ag_in = dram.tile([AGR, DBF], bf16, tag="ag_in")
            cn_dram = dram.tile([AGR * NCORES, DBF], bf16, tag="cn",
                                addr_space="Shared")
            cnts_dram = dram.tile([1, 1024], bf16, tag="cnts_dram")

            # zero the padded class rows of red_in once (classes 1000..1023)
            zero_t = constp.tile([128, RW], bf16, tag="zero_t")
            nc.vector.memset(zero_t[:], 0.0)
            nc.sync.dma_start(out=red_in[C:CPAD, :], in_=zero_t[:CPAD - C, :])

            f_tiles, lab_tiles = [], []

            with (tc.tile_pool(name="ohp", bufs=1) as ohp,
                  tc.tile_pool(name="psseg", bufs=1, space="PSUM") as psseg):
                # ---- phase 1: load + normalize f (bf16); one-hot (bf16) ----
                oh_tiles, x_tiles = [], []
                for b in range(NB):
                    r0 = b * 128
                    x_t = stream.tile([128, D], bf16, tag="xin", bufs=3,
                                      name=f"x{b}")
                    nc.sync.dma_start(out=x_t[:], in_=feat[r0:r0 + 128, :])
                    x_tiles.append(x_t)
                    lab_t = resid.tile([128, 1], i32, tag=f"lab{b}",
                                       name=f"lab{b}")
                    nc.sync.dma_start(out=lab_t[:], in_=labels[r0:r0 + 128, :])
                    lab_tiles.append(lab_t)
                for b in range(NB):
                    x_t = x_tiles[b]
                    ss = small.tile([128, 1], fp32, tag="ss")
                    scr = stream.tile([128, D], fp32, tag="scrB")
                    nc.scalar.activation(out=scr[:], in_=x_t[:],
                                         func=AF.Square, accum_out=ss[:])
                    nrm = small.tile([128, 1], fp32, tag="nrm")
                    nc.scalar.activation(out=nrm[:], in_=ss[:], func=AF.Sqrt)
                    nc.vector.tensor_scalar_max(nrm[:], nrm[:], 1e-12)
                    rin = small.tile([128, 1], fp32, tag="rin")
                    nc.vector.reciprocal(rin[:], nrm[:])
                    f_t = resid.tile([128, D], bf16, tag=f"f{b}",
                                     name=f"f{b}")
                    nc.vector.tensor_scalar_mul(f_t[:], x_t[:], rin[:, :1])
                    f_tiles.append(f_t)
                    lab_f = small.tile([128, 1], fp32, tag="labf")
                    nc.vector.tensor_copy(lab_f[:], lab_tiles[b][:])
                    oh_t = ohp.tile([128, C], bf16, tag=f"oh{b}",
                                    name=f"oh{b}")
                    nc.vector.tensor_scalar(
                        out=oh_t[:], in0=iota_t[:], scalar1=lab_f[:, :1],
                        scalar2=None, op0=ALU.is_equal)
                    oh_tiles.append(oh_t)

                # ---- phase 2a: counts row -> column 1024 of red_in ----
                ps_cnt = psseg.tile([1, 1024], fp32, tag="segcnt", bufs=1)
                for b in range(NB):
                    st, sp = (b == 0), (b == NB - 1)
                    for n0, nsz in ((0, 512), (512, C - 512)):
                        nc.tensor.matmul(ps_cnt[:1, n0:n0 + nsz],
                                         lhsT=ones_col[:, :1],
                                         rhs=oh_tiles[b][:, n0:n0 + nsz],
                                         start=st, stop=sp)
                cnts_f = small.tile([1, 1024], bf16, tag="cnts_f")
                nc.vector.memset(cnts_f[:], 0.0)
                nc.vector.tensor_copy(cnts_f[:1, 0:C], ps_cnt[:1, 0:C])
                nc.sync.dma_start(out=cnts_dram[:, :], in_=cnts_f[:1, :])
                cnts_cols = small.tile([128, NCCH], bf16, tag="cnts_cols")
                nc.sync.dma_start(
                    out=cnts_cols[:, :],
                    in_=cnts_dram[0:1, :].rearrange("a (q p) -> (a p) q",
                                                    q=NCCH))
                nc.sync.dma_start(
                    out=red_in[:, D:D + 1].rearrange("(q p) a -> p (q a)",
                                                     q=NCCH),
                    in_=cnts_cols[:, :])

                # ---- phase 2b: segment sums (bf16 one-hot matmul) ----
                for ci, (c0, csz) in enumerate(c_chunks):
                    ps = psseg.tile([128, D], fp32, tag="segsum", bufs=3)
                    for b in range(NB):
                        for dh in (0, 512):
                            nc.tensor.matmul(
                                ps[:csz, dh:dh + 512],
                                lhsT=oh_tiles[b][:, c0:c0 + csz],
                                rhs=f_tiles[b][:, dh:dh + 512],
                                start=(b == 0), stop=(b == NB - 1))
                    sums_f = stream.tile([128, D], bf16, tag="sums_f",
                                         bufs=2)
                    if ci % 2 == 0:
                        nc.scalar.copy(sums_f[:csz, :], ps[:csz, :])
                    else:
                        nc.vector.tensor_copy(sums_f[:csz, :], ps[:csz, :])
                    nc.sync.dma_start(out=red_in[c0:c0 + csz, 0:D],
                                      in_=sums_f[:csz, :])

            # ---- phase 3: ReduceScatter (core k owns classes 128k..) ----
            nc.gpsimd.collective_compute(
                "ReduceScatter", ALU.add,
                ins=[red_in.opt()], outs=[rs_out.opt()],
                replica_groups=[list(range(NCORES))])
            psmm = est.enter_context(
                tc.tile_pool(name="psmm", bufs=1, space="PSUM"))

            # ---- phase 4: momentum update of this core's 128 classes ----
            cen = stream.tile([128, D], fp32, tag="scrB")
            nc.sync.dma_start(out=cen[:, :], in_=centers_sh[:, :])
            rs_sb = stream.tile([128, RW], bf16, tag="rs_sb")
            nc.sync.dma_start(out=rs_sb[:, :], in_=rs_out[:, :])
            csq_col = small.tile([128, 1], fp32, tag="csq_col")
            cn_bf = resid.tile([128, DBF], bf16, tag="cn_bf")
            cntc = small.tile([128, 1], fp32, tag="cntc")
            nc.vector.tensor_scalar_max(cntc[:], rs_sb[:, D:D + 1], 1.0)
            rcv = small.tile([128, 1], fp32, tag="rcv")
            nc.vector.reciprocal(rcv[:], cntc[:])
            w = small.tile([128, 1], fp32, tag="w")
            nc.vector.tensor_scalar(out=w[:], in0=rs_sb[:, D:D + 1],
                                    scalar1=0.0, scalar2=1.0 - MOM,
                                    op0=ALU.is_gt, op1=ALU.mult)
            m = small.tile([128, 1], fp32, tag="m")
            nc.vector.tensor_tensor(out=m[:], in0=w[:], in1=rcv[:],
                                    op=ALU.mult)
            u = small.tile([128, 1], fp32, tag="u")
            nc.vector.tensor_scalar(out=u[:], in0=w[:], scalar1=-1.0,
                                    scalar2=1.0, op0=ALU.mult, op1=ALU.add)
            t1 = stream.tile([128, D], fp32, tag="scrC")
            nc.scalar.mul(t1[:], cen[:, :], u[:, :1])
            cn_t = stream.tile([128, D], fp32, tag="cn_t")
            nc.vector.scalar_tensor_tensor(
                out=cn_t[:, :], in0=rs_sb[:, 0:D], scalar=m[:, :1],
                in1=t1[:, :], op0=ALU.mult, op1=ALU.add)
            scr2 = stream.tile([128, D], bf16, tag="sqdump")
            nc.scalar.activation(out=scr2[:], in_=cn_t[:, :],
                                 func=AF.Square, accum_out=csq_col[:])
            nc.vector.tensor_copy(cn_bf[:, 0:D], cn_t[:, :])
            nc.vector.tensor_copy(cn_bf[:, D:D + 2].bitcast(fp32),
                                  csq_col[:, :])
            nc.vector.memset(cn_bf[:, D + 2:DBF], 0.0)
            nc.sync.dma_start(out=ag_in[0:128, :], in_=cn_bf[:, :])

            # local transposes of this core's CnT blocks -> ag2 payload
            stage = resid.tile([128, 1024], bf16, tag="stage")
            for dj in range(8):
                tpl = psmm.tile([128, 128], bf16, tag="tpl", bufs=2)
                nc.tensor.transpose(
                    out=tpl[:, :], in_=cn_bf[:, dj * 128:(dj + 1) * 128],
                    identity=ident_b[:, :])
                if dj % 2 == 0:
                    nc.scalar.copy(stage[:, dj * 128:(dj + 1) * 128],
                                   tpl[:, :])
                else:
                    nc.vector.tensor_copy(stage[:, dj * 128:(dj + 1) * 128],
                                          tpl[:, :])
            myT = resid.tile([128, 1024], bf16, tag="myT")
            nc.vector.tensor_scalar_mul(myT[:], stage[:, :], -2.0)
            csq_bf = small.tile([1, 128], bf16, tag="csq_bf")
            tpc = psmm.tile([1, 128], fp32, tag="tpc", bufs=1)
            nc.tensor.transpose(out=tpc[:1, :], in_=csq_col[:, :1],
                                identity=ident_f[:, :])
            nc.vector.tensor_copy(csq_bf[:1, :], tpc[:1, :])
            nc.sync.dma_start(out=ag_in[128:256, 0:1024], in_=stage[:, :])
            nc.sync.dma_start(out=ag_in[256:257, 0:128], in_=csq_bf[:1, :])

            # ---- phase 5: one merged AllGather ----
            nc.gpsimd.collective_compute(
                "AllGather", ALU.bypass,
                ins=[ag_in.opt()], outs=[cn_dram.opt()],
                replica_groups=[list(range(NCORES))])

            # ---- phase 2c: features_adv norms (gpsimd; overlap the RS) ----
            xa_tiles, rina_tiles = [], []
            for b in range(NB):
                r0 = b * 128
                xa_t = stream.tile([128, D], fp32, tag="scrA")
                nc.sync.dma_start(out=xa_t[:], in_=feat_adv[r0:r0 + 128, :])
                ssa = small.tile([128, 1], fp32, tag="ss")
                scr = stream.tile([128, D], fp32, tag="scrB")
                nc.scalar.activation(out=scr[:], in_=xa_t[:],
                                     func=AF.Square, accum_out=ssa[:])
                nrma = small.tile([128, 1], fp32, tag="nrm")
                nc.scalar.activation(out=nrma[:], in_=ssa[:], func=AF.Sqrt)
                nc.vector.tensor_scalar_max(nrma[:], nrma[:], 1e-12)
                rina = resid.tile([128, 1], fp32, tag=f"rina{b}",
                                  name=f"rina{b}")
                nc.vector.reciprocal(rina[:], nrma[:])
                rina_tiles.append(rina)
                xa_bf = resid.tile([128, D], bf16, tag=f"xa{b}",
                                   name=f"xa{b}")
                nc.vector.tensor_scalar_mul(xa_bf[:], xa_t[:], rina[:, :1])
                xa_tiles.append(xa_bf)


            # pairwise operands: CnT tiles + csq row from the stage rows.
            # stage row d, col dj*128+j  ==  CnT[dj*128+d, local class j]
            stage_v = cn_dram[:, :].rearrange(
                "(k r) j -> r k j", k=NCORES)[128:256, :, 0:1024].rearrange(
                "p k (dj j) -> p k dj j", dj=8)
            cnt_sb = []
            for dj in range(8):
                ct = resid.tile([128, CPAD], bf16, tag=f"cnt{dj}",
                                name=f"cnt_sb{dj}")
                eng = nc.sync if dj % 2 == 0 else nc.scalar
                eng.dma_start(out=ct[:], in_=stage_v[:, :, dj, :])
                cnt_sb.append(ct)
            csq_row = constp.tile([1, 1024], bf16, tag="csq_row")
            nc.sync.dma_start(
                out=csq_row[:],
                in_=cn_dram[:, :].rearrange("(k r) j -> r k j",
                                            k=NCORES)[256:257, :, 0:128])

            # ---- phase 6: intra losses via fused dots + gathered csq ----
            lg_tiles = []
            for b in range(NB):
                lg_t = resid.tile([128, 1], i32, tag=f"lg{b}",
                                  name=f"lg{b}")
                nc.sync.dma_start(out=lg_t[:], in_=labels_g[:, b:b + 1])
                lg_tiles.append(lg_t)

            # ---- phase 7: pairwise inter loss (rows = this core's own
            # 128 classes; zero pad rows contribute exactly 0) ----
            g_ps = psmm.tile([128, C], fp32, tag="gmm", bufs=1)
            for dj in range(8):
                for n0, nsz in ((0, 512), (512, C - 512)):
                    nc.tensor.matmul(g_ps[:, n0:n0 + nsz],
                                     lhsT=myT[:, dj * 128:(dj + 1) * 128],
                                     rhs=cnt_sb[dj][:, n0:n0 + nsz],
                                     start=(dj == 0), stop=False)
            for n0, nsz in ((0, 512), (512, C - 512)):
                nc.tensor.matmul(g_ps[:, n0:n0 + nsz], lhsT=ones_row[:1, :],
                                 rhs=csq_row[:1, n0:n0 + nsz],
                                 start=False, stop=True)
            d2b = stream.tile([128, C], fp32, tag="scrB")
            nc.vector.tensor_scalar(
                out=d2b[:], in0=g_ps[:, :],
                scalar1=csq_col[:, :1],
                scalar2=0.0, op0=ALU.add, op1=ALU.max)
            dst = stream.tile([128, C], fp32, tag="scrC")
            nc.scalar.activation(out=dst[:], in_=d2b[:], func=AF.Sqrt)
            term = stream.tile([128, C], fp32, tag="scrA")
            inter_rows = small.tile([128, 1], fp32, tag="inter_rows")
            nc.scalar.activation(out=term[:], in_=dst[:],
                                 func=AF.Relu, bias=1.0, scale=-1.0,
                                 accum_out=inter_rows[:])


            dots_f = resid.tile([128, NB], fp32, tag="dots_f")
            gq = resid.tile([128, NB], fp32, tag="gq")
            ssa_t = small.tile([128, NB], fp32, tag="ssa_t")
            for b in range(NB):
                g_t = stream.tile([128, DBF], bf16, tag="gat", bufs=6)
                nc.gpsimd.indirect_dma_start(
                    out=g_t[:], out_offset=None, in_=cn_dram[:, :],
                    in_offset=bass.IndirectOffsetOnAxis(
                        ap=lg_tiles[b][:, :1], axis=0))
                nc.vector.tensor_copy(gq[:, b:b + 1],
                                      g_t[:, D:D + 2].bitcast(fp32))
                prodf = stream.tile([128, D], bf16, tag="pdump", bufs=3)
                nc.vector.scalar_tensor_tensor(
                    out=prodf[:], in0=f_tiles[b][:], scalar=1.0,
                    in1=g_t[:, 0:D], op0=ALU.mult, op1=ALU.mult,
                    accum_out=dots_f[:, b:b + 1])
                da_t = stream.tile([128, D], bf16, tag="pdump", bufs=3)
                nc.vector.tensor_tensor(out=da_t[:], in0=xa_tiles[b][:],
                                        in1=g_t[:, 0:D],
                                        op=ALU.subtract)
                scra = stream.tile([128, D], bf16, tag="sqd2", bufs=3)
                nc.scalar.activation(out=scra[:], in_=da_t[:],
                                     func=AF.Square,
                                     accum_out=ssa_t[:, b:b + 1])
            # ssf = 1 + gq - 2*dots_f
            gq1 = small.tile([128, NB], fp32, tag="gq1")
            nc.vector.tensor_scalar(out=gq1[:], in0=gq[:], scalar1=1.0,
                                    scalar2=None, op0=ALU.add)
            ssf_t = small.tile([128, NB], fp32, tag="ssf_t")
            nc.vector.scalar_tensor_tensor(
                out=ssf_t[:], in0=dots_f[:], scalar=-2.0, in1=gq1[:],
                op0=ALU.mult, op1=ALU.add)
            nc.vector.tensor_scalar_max(ssf_t[:], ssf_t[:], 0.0)

            dist_f = small.tile([128, NB], fp32, tag="dist_f")
            nc.scalar.activation(out=dist_f[:], in_=ssf_t[:], func=AF.Sqrt)
            dist_a = small.tile([128, NB], fp32, tag="dist_a")
            nc.scalar.activation(out=dist_a[:], in_=ssa_t[:], func=AF.Sqrt)
            ir_f = small.tile([128, 1], fp32, tag="ir_f")
            nc.vector.tensor_reduce(out=ir_f[:], in_=dist_f[:], axis=AX.X,
                                    op=ALU.add)
            ir_a = small.tile([128, 1], fp32, tag="ir_a")
            nc.vector.tensor_reduce(out=ir_a[:], in_=dist_a[:], axis=AX.X,
                                    op=ALU.add)
            intra_rows = small.tile([128, 1], fp32, tag="intra_rows")
            nc.vector.tensor_add(intra_rows[:], ir_f[:], ir_a[:])

            # ---- phase 8: final reduce + tiny AllReduce + formula ----
            partials = small.tile([128, 2], fp32, tag="partials")
            nc.vector.memset(partials[:], 0.0)
            nc.vector.tensor_copy(partials[:, 0:1], intra_rows[:])
            nc.vector.tensor_copy(partials[:, 1:2], inter_rows[:, :])
            pr = small.tile([1, 2], fp32, tag="pr")
            nc.gpsimd.tensor_reduce(out=pr[:1, :], in_=partials[:, :],
                                    axis=AX.C, op=ALU.add)
            nc.sync.dma_start(out=out[0:1, 0:2], in_=pr[:1, :])

    nc.compile()
    return nc


def _get_nc():
    if "nc" not in _state:
        _state["nc"] = _build()
    return _state["nc"]


def kernel(features, features_adv, centers, labels):
    from concourse import bass_utils

    nc = _get_nc()
    import ml_dtypes
    features_bf = np.ascontiguousarray(
        np.asarray(features, dtype=np.float32).astype(ml_dtypes.bfloat16))
    features_adv = np.ascontiguousarray(np.asarray(features_adv,
                                                   dtype=np.float32))
    centers_np = np.asarray(centers, dtype=np.float32)
    centers_pad = np.zeros((CPAD, D), dtype=np.float32)
    centers_pad[:C] = centers_np
    labels_i32 = np.ascontiguousarray(
        np.asarray(labels).astype(np.int32).reshape(B, 1))
    # gather-row remap: class c lives at AG row 257*(c//128) + c%128
    labels_gr = (AGR * (labels_i32 >> 7) + (labels_i32 & 127)).astype(
        np.int32).reshape(B)

    in_maps = []
    for k in range(NCORES):
        sl = slice(k * BLOC, (k + 1) * BLOC)

        in_maps.append({
            "features": features_bf[sl],
            "features_adv": features_adv[sl],
            "centers_sh": np.ascontiguousarray(
                centers_pad[k * 128:(k + 1) * 128]),
            "labels": labels_i32[sl],
            "labels_g": np.ascontiguousarray(
                labels_gr[sl].reshape(NB, 128).T),
        })

    res = bass_utils.run_bass_kernel_spmd(
        nc, in_maps, core_ids=list(range(NCORES)),
        trace=bool(int(os.environ.get("AFD_TRACE", "0"))))
    _state["last_results"] = res
    parts = np.stack([res.results[k]["out"][0] for k in range(NCORES)])
    intra_sum = float(parts[:, 0].sum())
    inter_sum = float(parts[:, 1].sum())
    val = intra_sum / B - 0.25 * (inter_sum - C) / N_PAIRS
    return np.asarray(np.float32(val))
